# revision 1
# baseline (speedup 1.0000x reference)
"""MoE FFN with hierarchical KV router — Trainium2 Bass kernel (8 NeuronCores).

Strategy (expert-parallel, per the sharding hint):
  * Host computes the router (l2-norm scores -> softmax over EPB=4 -> top-2 ->
    combine weights) and dispatches tokens by global expert id — the
    control-plane "all-to-all by gid" of the sharding step.
  * All FFN FLOPs run on device. Work is packed into uniform "segments",
    each segment = (W1, b1, W2, b2, CAP gathered tokens, per-token scale):
        out_seg = scale * (relu(x @ W1 + b1) @ W2 + b2)
    - one segment per expert chunk (scale = sigmoid(gate_logit) * combine_w)
    - the shared dense FFN is packed as plain segments too (scale = 1)
    Each of the 8 cores runs G segments (same program, different data).
  * Host un-shards by gathering each token's 3 segment rows (2 expert + 1
    shared) and combining them with the per-token weights:
        y[tok] = gate*w0*row0 + gate*w1*row1 + row_shared

Device kernel: raw Bass (explicit engine streams + semaphores), float32r
matmuls (fp32 rounded to 11 mantissa bits, full-rate PE) with activations kept
transposed ([feature, token]) so both layers use weights as the stationary
operand and no on-device transposes are needed. Per segment, inputs arrive as
four contiguous DMA pieces (A: xt+b1+W1-lo, B: W1-hi, C1: b2+W2-lo, C2: W2-hi)
into NBUF=3 rotating SBUF buffers so all input DMAs issue back-to-back and
matmuls start as soon as piece A lands; mm2 runs in two half-passes over k2 so
only the C2-gated half is exposed after the last DMA byte; outputs stream out
per 128-row group from GpSimd.

Blob layout per segment (per partition p, 4-byte cols):
  A:  [XTO, B1O)  xt  col k*CAP + t   = x[tok t, k*128+p]        (f32r)
      [B1O, W1O)  b1  col m  = b1[m*128+p]                       (f32)
      [W1O, AEND) w1  col m*512 + k*128 + q = W1[k*128+p, m*128+q], m<4
  B:  [AEND,BEND) w1 cols for m>=4                               (f32r)
  C1: [B2O, W2M)  b2  col m2 = b2[m2*128+p] (f32); w2 k2<4       (f32r)
  C2: [W2M, COLS) w2 cols for k2>=4, col k2*C + c = W2[k2*128+p, c]
out[g] = [128, KC*CAP]: col m2*CAP + t = FFN(x)[t, m2*128+p] (unweighted)
"""
import sys

if "/opt/trn_rl_repo" not in sys.path:
    sys.path.insert(0, "/opt/trn_rl_repo")

import numpy as np

N_BUCKET, EPB, TOPK, TAU = 4, 4, 2, 1.0
C, H = 512, 1024
E = N_BUCKET * EPB
KC, KH = C // 128, H // 128  # contraction blocks: 4, 8
N_CORES = 8

_BUILD_CACHE = {}


def _offsets(CAP):
    XTO = 0
    B1O = XTO + KC * CAP
    W1O = B1O + KH            # w1 cols: m*512 + k*128 + q (m-major!)
    AEND = W1O + KC * H // 2  # piece A = [0, AEND): xt, b1, w1 m<4
    BEND = W1O + KC * H       # piece B = [AEND, BEND): w1 m>=4
    B2O = BEND
    W2O = B2O + KC
    W2M = W2O + KH * C // 2   # piece C1 = [B2O, W2M): b2, w2 k2<4
    COLS = W2O + KH * C       # piece C2 = [W2M, COLS): w2 k2>=4
    return XTO, B1O, W1O, W2O, B2O, COLS, AEND, BEND, W2M


def _build_program(G, CAP):
    """Raw-bass program: G segments of CAP tokens through a C->H->C relu FFN."""
    from contextlib import ExitStack

    import concourse.bass as bass
    import concourse.mybir as mybir

    f32 = mybir.dt.float32
    f32r = mybir.dt.float32r
    XTO, B1O, W1O, W2O, B2O, COLS, AEND, BEND, W2M = _offsets(CAP)
    NBUF = min(G, 3)

    nc = bass.Bass("TRN2", target_bir_lowering=False, debug=False)
    blob = nc.declare_dram_parameter("blob", [G, 128, COLS], f32r, isOutput=False)
    out = nc.declare_dram_parameter("out", [G, 128, KC * CAP], f32, isOutput=True)

    def w1col(m, k):
        return W1O + m * 512 + k * 128

    with ExitStack() as ctx:
        BL = [ctx.enter_context(nc.sbuf_tensor(f"bl{i}", [128, COLS], f32r)) for i in range(NBUF)]
        H1 = [ctx.enter_context(nc.sbuf_tensor(f"h1_{i}", [128, KH * CAP], f32r)) for i in range(2)]
        OT = [ctx.enter_context(nc.sbuf_tensor(f"ot{i}", [128, KC * CAP], f32)) for i in range(2)]
        PS = [ctx.enter_context(nc.psum_tensor(f"ps{i}", [128, CAP], f32)) for i in range(8)]
        inA = [ctx.enter_context(nc.semaphore(f"inA{i}")) for i in range(NBUF)]
        inB = [ctx.enter_context(nc.semaphore(f"inB{i}")) for i in range(NBUF)]
        inC1 = [ctx.enter_context(nc.semaphore(f"inC1_{i}")) for i in range(NBUF)]
        inC2 = [ctx.enter_context(nc.semaphore(f"inC2_{i}")) for i in range(NBUF)]
        outS = [ctx.enter_context(nc.semaphore(f"outS{i}")) for i in range(2)]
        pe1 = ctx.enter_context(nc.semaphore("pe1"))
        pe2 = ctx.enter_context(nc.semaphore("pe2"))
        act1 = ctx.enter_context(nc.semaphore("act1"))
        dve1 = ctx.enter_context(nc.semaphore("dve1"))
        block = ctx.enter_context(nc.Block(no_gpsimd_drain=True))

        @block.sync
        def _(sync):
            def issue_blob(g):
                if g >= NBUF:
                    # blob slot g%NBUF recycle: all readers of segment g-NBUF done
                    sync.wait_ge(pe2, 4 * (g - NBUF + 1))
                    sync.wait_ge(act1, 8 * (g - NBUF + 1))
                    sync.wait_ge(dve1, 4 * (g - NBUF + 1))
                bl = BL[g % NBUF][:]
                sl = g % NBUF
                sync.dma_start(out=bl[:, XTO:AEND], in_=blob[g][:, XTO:AEND]).then_inc(inA[sl], 16)
                sync.dma_start(out=bl[:, AEND:BEND], in_=blob[g][:, AEND:BEND]).then_inc(inB[sl], 16)
                sync.dma_start(out=bl[:, B2O:W2M], in_=blob[g][:, B2O:W2M]).then_inc(inC1[sl], 16)
                sync.dma_start(out=bl[:, W2M:COLS], in_=blob[g][:, W2M:COLS]).then_inc(inC2[sl], 16)

            for g in range(G):
                issue_blob(g)
            n_even = (G + 1) // 2
            n_odd = G // 2
            sync.wait_ge(outS[0], 16 * KC * n_even)
            sync.wait_ge(outS[1], 16 * KC * n_odd)

        @block.gpsimd
        def _(gpsimd):
            for g in range(G):
                for m2 in range(KC):
                    gpsimd.wait_ge(dve1, 4 * g + m2 + 1)
                    gpsimd.dma_start(
                        out=out[g][:, m2 * CAP: (m2 + 1) * CAP],
                        in_=OT[g % 2][:, m2 * CAP: (m2 + 1) * CAP],
                    ).then_inc(outS[g % 2], 16)

        @block.tensor
        def _(tensor):
            for g in range(G):
                sl = g % NBUF
                bl = BL[sl][:]
                h1 = H1[g % 2][:]
                # mm1: h1T[m] = W1[:,m]^T @ xT   (accumulate over KC chunks)
                tensor.wait_ge(inA[sl], 16 * (g // NBUF + 1))
                for m in range(KH):
                    if m == 4:
                        tensor.wait_ge(inB[sl], 16 * (g // NBUF + 1))
                    if m >= 4:
                        tensor.wait_ge(act1, 8 * g + (m - 4) + 1)  # ps bank m%4 free
                    for k in range(KC):
                        mm = nc.tensor.matmul(
                            PS[m % 4][:],
                            lhsT=bl[:, w1col(m, k): w1col(m, k) + 128],
                            rhs=bl[:, XTO + k * CAP: XTO + (k + 1) * CAP],
                            start=(k == 0),
                            stop=(k == KC - 1),
                        )
                    mm.then_inc(pe1, 1)
                # mm2: outT[m2] = W2[:,m2]^T @ h1T, two half-passes over k2
                tensor.wait_ge(inC1[sl], 16 * (g // NBUF + 1))
                for m2 in range(KC):
                    if g >= 1:
                        tensor.wait_ge(dve1, 4 * (g - 1) + m2 + 1)  # ps bank 4+m2 free
                    for k2 in range(KH // 2):
                        if m2 == 0:
                            tensor.wait_ge(act1, 8 * g + k2 + 1)  # h1[k2] ready
                        nc.tensor.matmul(
                            PS[4 + m2][:],
                            lhsT=bl[:, W2O + k2 * C + m2 * 128: W2O + k2 * C + (m2 + 1) * 128],
                            rhs=h1[:, k2 * CAP: (k2 + 1) * CAP],
                            start=(k2 == 0),
                            stop=False,
                        )
                tensor.wait_ge(inC2[sl], 16 * (g // NBUF + 1))
                for m2 in range(KC):
                    for k2 in range(KH // 2, KH):
                        if m2 == 0:
                            tensor.wait_ge(act1, 8 * g + k2 + 1)  # h1[k2] ready
                        mm = nc.tensor.matmul(
                            PS[4 + m2][:],
                            lhsT=bl[:, W2O + k2 * C + m2 * 128: W2O + k2 * C + (m2 + 1) * 128],
                            rhs=h1[:, k2 * CAP: (k2 + 1) * CAP],
                            start=False,
                            stop=(k2 == KH - 1),
                        )
                    mm.then_inc(pe2, 1)

        @block.scalar
        def _(scalar):
            for g in range(G):
                bl = BL[g % NBUF][:]
                h1 = H1[g % 2][:]
                for m in range(KH):
                    if g >= 2 and m == 0:
                        scalar.wait_ge(pe2, 4 * (g - 1))  # h1 slot recycle
                    scalar.wait_ge(pe1, 8 * g + m + 1)
                    nc.scalar.activation(
                        h1[:, m * CAP: (m + 1) * CAP],
                        PS[m % 4][:],
                        mybir.ActivationFunctionType.Relu,
                        bias=bl[:, B1O + m: B1O + m + 1].bitcast(f32),
                    ).then_inc(act1, 1)

        @block.vector
        def _(vector):
            for g in range(G):
                bl = BL[g % NBUF][:]
                ot = OT[g % 2][:]
                for m2 in range(KC):
                    if g >= 2 and m2 == 0:
                        vector.wait_ge(outS[g % 2], 16 * KC * (g // 2))  # o_t slot recycle
                    vector.wait_ge(pe2, 4 * g + m2 + 1)
                    nc.vector.tensor_scalar_add(
                        ot[:, m2 * CAP: (m2 + 1) * CAP],
                        PS[4 + m2][:],
                        bl[:, B2O + m2: B2O + m2 + 1].bitcast(f32),
                    ).then_inc(dve1, 1)

    return nc


def _round_f32r(a):
    """Round float32 array to fp32r (round-to-nearest-even at mantissa bit 12)."""
    u = np.ascontiguousarray(a, np.float32).view(np.uint32)
    lsb = (u >> 12) & 1
    r = (u + 0x7FF + lsb) & 0xFFFFF000
    return r.view(np.float32)


def _route(x2, bucket, expert_key):
    """Host router in float64. Returns gid (N,2), combine weights (N,2)."""
    hn = x2 / np.maximum(np.linalg.norm(x2, axis=-1, keepdims=True), 1e-12)
    keys = expert_key / np.maximum(
        np.linalg.norm(expert_key, axis=-1, keepdims=True), 1e-12
    )
    kb = keys[bucket]  # (N, EPB, C)
    score = np.einsum("nc,nec->ne", hn, kb) / max(TAU, 1e-6)
    score -= score.max(axis=-1, keepdims=True)
    p = np.exp(score)
    p /= p.sum(axis=-1, keepdims=True)
    local = np.argsort(-p, axis=-1, kind="stable")[:, :TOPK]  # (N, 2)
    topv = np.take_along_axis(p, local, axis=-1)
    w = topv / (topv.sum(axis=-1, keepdims=True) + 1e-9)
    gid = bucket[:, None] * EPB + local
    return gid, w


def kernel(**inputs):
    from concourse.bass_utils import run_bass_kernel_spmd

    x = np.asarray(inputs["x"], dtype=np.float32)
    op_id = np.asarray(inputs["op_id"]).astype(np.int64)
    expert_key = np.asarray(inputs["expert_key"], dtype=np.float64)
    sW1 = np.asarray(inputs["sW1"], dtype=np.float32)
    sb1 = np.asarray(inputs["sb1"], dtype=np.float32)
    sW2 = np.asarray(inputs["sW2"], dtype=np.float32)
    sb2 = np.asarray(inputs["sb2"], dtype=np.float32)
    eW1 = np.asarray(inputs["eW1"], dtype=np.float32)
    eb1 = np.asarray(inputs["eb1"], dtype=np.float32)
    eW2 = np.asarray(inputs["eW2"], dtype=np.float32)
    eb2 = np.asarray(inputs["eb2"], dtype=np.float32)
    gate_logit = float(np.asarray(inputs["gate_logit"]))

    B, T, Cc = x.shape
    assert Cc == C
    N = B * T
    x2 = x.reshape(N, C)
    bucket = np.clip(op_id.reshape(-1), 0, N_BUCKET - 1)

    gid, w = _route(x2.astype(np.float64), bucket, expert_key)
    gate = 1.0 / (1.0 + np.exp(-gate_logit))

    # ---- pack work into segments of CAP token slots --------------------
    flat_gid = gid.reshape(-1)  # (N*2,) ; slot i -> token i//2
    sorted_slots = np.argsort(flat_gid, kind="stable")
    counts = np.bincount(flat_gid, minlength=E)

    # choose CAP: minimize G = ceil(S/8), then CAP
    best = None
    for cap in range(256, 513, 32):
        S = int(sum(-(-c // cap) for c in counts if c > 0)) + -(-N // cap)
        Gc = -(-S // N_CORES)
        key = (Gc, cap)
        if best is None or key < best[:2]:
            best = (Gc, cap, S)
    G, CAP, S = best
    S_pad = G * N_CORES
    XTO, B1O, W1O, W2O, B2O, COLS, AEND, BEND, W2M = _offsets(CAP)

    blob = np.zeros((S_pad, 128, COLS), np.float32)
    slot_flat = np.zeros((3, N), np.int64)  # each token: 2 expert rows + 1 shared row
    x2T_r = _round_f32r(x2.T)  # (C, N)

    def fill_segment(s, w1_, b1_, w2_, b2_, tok_idx):
        n = len(tok_idx)
        # w1 m-major: col m*512 + k*128 + q = W1[k*128+p, m*128+q]
        w1m = w1_.reshape(KC, 128, KH, 128).transpose(1, 2, 0, 3).reshape(128, KC * H)
        blob[s, :, W1O:BEND] = w1m
        blob[s, :, W2O:COLS] = w2_.reshape(KH, 128, C).transpose(1, 0, 2).reshape(128, KH * C)
        xg = x2T_r[:, tok_idx]  # (C, n)
        blob[s, :, XTO:B1O].reshape(128, KC, CAP)[:, :, :n] = (
            xg.reshape(KC, 128, n).transpose(1, 0, 2)
        )
        blob[s, :, B1O:W1O] = b1_.reshape(KH, 128).T
        blob[s, :, B2O:W2O] = b2_.reshape(KC, 128).T

    ew1r = _round_f32r(eW1)
    ew2r = _round_f32r(eW2)
    sw1r = _round_f32r(sW1)
    sw2r = _round_f32r(sW2)

    s = 0
    pos = 0
    for e in range(E):
        cnt = int(counts[e])
        slots_e = sorted_slots[pos: pos + cnt]
        pos += cnt
        for lo in range(0, cnt, CAP):
            chunk = slots_e[lo: lo + CAP]
            toks = chunk // TOPK
            fill_segment(s, ew1r[e], eb1[e], ew2r[e], eb2[e], toks)
            slot_flat[chunk % TOPK, toks] = s * CAP + np.arange(len(chunk))
            s += 1
    for lo in range(0, N, CAP):
        toks = np.arange(lo, min(lo + CAP, N))
        fill_segment(s, sw1r, sb1, sw2r, sb2, toks)
        slot_flat[2, toks] = s * CAP + np.arange(len(toks))
        s += 1
    assert s == S <= S_pad

    # ---- compile + run on the 8 cores ----------------------------------
    key = (G, CAP)
    if key not in _BUILD_CACHE:
        _BUILD_CACHE[key] = _build_program(G, CAP)
    nc = _BUILD_CACHE[key]

    in_maps = [{"blob": blob[c * G: (c + 1) * G]} for c in range(N_CORES)]

    import os

    trace = bool(os.environ.get("BASS_TRACE"))
    res = run_bass_kernel_spmd(
        nc,
        in_maps,
        core_ids=list(range(N_CORES)),
        trace=trace,
        trace_cores=list(range(N_CORES)) if trace else None,
    )
    global LAST_EXEC_NS, LAST_RESULTS
    LAST_EXEC_NS = res.exec_time_ns
    LAST_RESULTS = res

    # ---- un-shard: gather each token's 3 rows and add ------------------
    # core output (G, 128, KC*CAP): col m2*CAP+t, C index = m2*128+p
    allout = np.empty((S_pad * CAP, C), np.float32)
    for c in range(N_CORES):
        o = np.asarray(res.results[c]["out"]).reshape(G, 128, KC, CAP)
        o = o.transpose(0, 3, 2, 1).reshape(G * CAP, C)  # token-major
        allout[c * G * CAP: (c + 1) * G * CAP] = o

    wf = (gate * w).astype(np.float32)  # (N, 2) combine weights
    y = (
        allout[slot_flat[0]] * wf[:, 0:1]
        + allout[slot_flat[1]] * wf[:, 1:2]
        + allout[slot_flat[2]]
    )
    return y.reshape(B, T, C)


LAST_EXEC_NS = None
LAST_RESULTS = None

